# revision 24
# baseline (speedup 1.0000x reference)
"""Trainium2 Bass kernel for nn_AttnBlock (GroupNorm + single-head spatial
attention + projection + residual), sharded over 8 NeuronCores.

v2: fp8e4 DoubleRow matmuls throughout (256-row contraction per pass).

Strategy (sequence-parallel over queries, K/V side replicated):
  - x [1,512,8,32,32] -> x2d [C=512, N=8192]. Host ships x in TWO fp8
    layouts: c-on-partitions (scores stationary) and m-on-partitions
    (PV stationary) -- no on-device transposes.
  - GroupNorm folded into weights: hn = A*x + B per channel. Group stats
    computed on device: channel sums / sum-squares via ones-stationary
    DoubleRow matmuls over the m-layout tiles (squares split across
    Pool/Act/DVE), group reduce + broadcast via tiny matmuls.
  - Scores computed transposed: S^T[m,q] = x^T (A .* Wk^T Q); K bias
    cancels in softmax. P = exp(scale*S^T) written directly as fp8 by the
    Act engine; max-subtraction skipped (scores O(1)).
  - ho[c,q] = sum_m x[c,m] P[m,q] accumulates in PSUM across the WHOLE
    m loop (no SBUF flushes); row-sums r[q] via ones-stationary fp8
    matmuls into a dedicated PSUM bank. Softmax normalization applied to
    ho (commutes with the V/proj GEMMs); V and proj biases fold into a
    single output bias. The Vfold/proj tail of query-block 0 is
    interleaved into query-block 1's m loop to avoid a pipeline bubble.
  - Weights are pre-scaled by 32 on host (fp8 subnormal avoidance) and
    rescaled in the PSUM->SBUF copies; all fp8 scale factors cancel via
    compile-time constants.
  - Each core computes its 1024-query slice; host gathers slices.
"""
import sys
import numpy as np

sys.path.insert(0, "/opt/trn_rl_repo")

import ml_dtypes
import concourse.bacc as bacc
import concourse.tile as tile
from concourse import mybir
from concourse.bass_utils import run_bass_kernel_spmd

F32 = mybir.dt.float32
BF16 = mybir.dt.bfloat16
FP8 = mybir.dt.float8e4
E4 = ml_dtypes.float8_e4m3
AF = mybir.ActivationFunctionType
ALU = mybir.AluOpType
DR = mybir.MatmulPerfMode.DoubleRow

N_CORES = 8
C = 512            # channels
M = 8192           # tokens (8*32*32)
QS = M // N_CORES  # queries per core (1024)
QB = 512           # query block
NQB = QS // QB     # 2
NPAIR = M // 256   # 32 m-pairs (256 tokens each)
NG = 16            # groupnorm groups
EPS = 1e-6
SCALE = float(C) ** -0.5

WS = 32.0    # host fp8 weight scale
BS = 256.0   # B-vector fp8 scale
QKS = 4.0    # qk fp8 scale
HOS = 32.0   # normalized-ho fp8 scale
HVS = 64.0   # hv fp8 scale


def build_nc(reps=1):
    nc = bacc.Bacc("TRN2", target_bir_lowering=False, debug=False,
                   num_devices=N_CORES)

    def din(name, shape, dtype=F32):
        return nc.dram_tensor(name, shape, dtype, kind="ExternalInput").ap()

    x8 = din("x8", [128, 2, 2, M], FP8)       # x[c,m], c = co*256+i*128+p
    xt8 = din("xt8", [128, NPAIR, 2, C], FP8)  # x[c, t*256+i*128+p]
    xq8 = din("xq8", [128, 2, 2, QS], FP8)     # query slice, c-layout
    x_res = din("x_res", [C, QS])              # residual slice fp32
    wq8 = din("wq8", [128, 2, 2, C], FP8)      # 32*Wq[o, cin], cin on part
    wk8 = din("wk8", [128, 2, 2, C], FP8)      # 32*Wk[o, c], o on part
    wv8 = din("wv8", [128, 2, 2, C], FP8)      # 32*Wv[o, cmid], cmid on part
    wp8 = din("wp8", [128, 2, 2, C], FP8)      # 32*Wp[o, cmid], cmid on part
    bq = din("bq", [C])
    bv = din("bv", [C])
    bp = din("bp", [C])
    gamma = din("gamma", [C])
    beta = din("beta", [C])
    ones8 = din("ones8", [128, 2, 128], FP8)
    one1 = din("one1", [1, 1])
    ones_1 = din("ones_1", [1, 128])
    e2 = din("e2", [16, 128])                  # E2[g,p] = (g%4 == p//32)
    gmask = din("gmask", [16, 4])              # gmask[g,cc] = (g//4 == cc)
    out = nc.dram_tensor("out", [C, QS], F32, kind="ExternalOutput").ap()

    xrv = x_res.rearrange("(oc p) n -> p oc n", p=128)
    outv = out.rearrange("(oc p) n -> p oc n", p=128)

    def vec1(ap):  # [C] -> [128, 4]
        return ap.rearrange("(cc p) -> p cc", p=128)

    with tile.TileContext(nc) as tc:
        import contextlib
        ctx = contextlib.ExitStack()
        with ctx:
            res = ctx.enter_context(tc.tile_pool(name="res", bufs=1))
            sqp = ctx.enter_context(tc.tile_pool(name="sqp", bufs=8))
            pgr = ctx.enter_context(tc.tile_pool(name="pgr", bufs=4))
            hop = ctx.enter_context(tc.tile_pool(name="hop", bufs=3))
            dmy = ctx.enter_context(tc.tile_pool(name="dmy", bufs=6))
            smal = ctx.enter_context(tc.tile_pool(name="smal", bufs=1))
            ps_sc = ctx.enter_context(tc.tile_pool(name="ps_sc", bufs=3, space="PSUM"))
            ps_ho = ctx.enter_context(tc.tile_pool(name="ps_ho", bufs=1, space="PSUM"))
            ps_r = ctx.enter_context(tc.tile_pool(name="ps_r", bufs=1, space="PSUM"))
            ps_x = ps_r

            # ---- resident tiles -------------------------------------------
            x8_sb = res.tile([128, 2, 2, M], FP8)
            xt8_sb = res.tile([128, NPAIR, 2, C], FP8)
            xq8_sb = res.tile([128, 2, 2, QS], FP8)
            q8_sb = res.tile([128, 2, 2, QS], FP8)
            qk8_sb = res.tile([128, 2, 2, QS], FP8)
            w_sb = {}
            for nm in ("q", "k", "v", "p"):
                w_sb[nm] = res.tile([128, 2, 2, C], FP8, name=f"w_{nm}",
                                    tag=f"w_{nm}")
            xres_sb = res.tile([128, 4, QS], F32)
            cvec = {}
            for nm in ("bq", "bv", "bp", "gamma", "beta"):
                cvec[nm] = res.tile([128, 4], F32, name=f"cv_{nm}",
                                    tag=f"cv_{nm}")
            ones8_sb = res.tile([128, 2, 128], FP8)
            one1_sb = res.tile([1, 1], F32)
            ones1_sb = res.tile([1, 128], F32)
            e2_sb = res.tile([16, 128], F32)
            gmask_sb = res.tile([16, 4], F32)

            def body():
                import os as _os
                _lvl = {"A": 0, "Q": 1, "B": 2, "P": 3}[
                    _os.environ.get("K2PHASE", "P")]
                # ======== DMA in ========================================
                # sync queue: xt8 (16 chunks, feeds stats pipeline), then x8
                for h in range(16):
                    nc.sync.dma_start(xt8_sb[:, h * 2:(h + 1) * 2],
                                      xt8[:, h * 2:(h + 1) * 2])
                for hf in range(2):
                    for co in range(2):
                        msl = slice(hf * (M // 2), (hf + 1) * (M // 2))
                        nc.sync.dma_start(x8_sb[:, co, :, msl],
                                          x8[:, co, :, msl])
                # gpsimd queue: consts + weights + xq (xres after squares)
                nc.gpsimd.dma_start(ones8_sb[:], ones8)
                nc.gpsimd.dma_start(one1_sb[:], one1)
                nc.gpsimd.dma_start(ones1_sb[:], ones_1)
                nc.gpsimd.dma_start(e2_sb[:], e2)
                nc.gpsimd.dma_start(gmask_sb[:], gmask)
                for nm, t in (("bq", bq), ("bv", bv), ("bp", bp),
                              ("gamma", gamma), ("beta", beta)):
                    nc.gpsimd.dma_start(cvec[nm][:], vec1(t))
                # warm the Act tables (Square + Ln/Exp) off the critical path
                warm8 = smal.tile([128, 2, 2], FP8, tag="warm8", bufs=1)
                nc.scalar.activation(out=warm8[:], in_=ones8_sb[:, :, 0:2],
                                     func=AF.Square)
                warm1 = smal.tile([1, 1], F32, tag="warm1", bufs=1)
                nc.scalar.activation(out=warm1[:], in_=one1_sb[:], func=AF.Ln)
                nc.scalar.activation(out=warm1[:], in_=warm1[:], func=AF.Exp)

                # ======== Phase A: group stats ==========================
                # squares of xt8 tiles split across Pool/Act/DVE; sums and
                # sumsq matmuls interleaved per pair so the square-tile pool
                # never stalls the sums stream.
                # sums_ps and sumsq_ps accumulate concurrently and must live
                # in different banks: sumsq goes to the sc pool (pre-loop).
                sums_ps = ps_r.tile([128, C], F32, tag="r")
                sumsq_ps = ps_sc.tile([128, C], F32, tag="sc")
                for t in range(NPAIR):
                    sq = sqp.tile([128, 2, C], FP8, tag="sq", name=f"sq{t}")
                    if t % 2 == 0:
                        nc.vector.tensor_tensor(out=sq[:], in0=xt8_sb[:, t],
                                                in1=xt8_sb[:, t], op=ALU.mult)
                    else:
                        nc.scalar.activation(out=sq[:], in_=xt8_sb[:, t],
                                             func=AF.Square)
                    nc.tensor.matmul(sums_ps[:], ones8_sb[:], xt8_sb[:, t],
                                     start=(t == 0), stop=(t == NPAIR - 1),
                                     perf_mode=DR)
                    nc.tensor.matmul(sumsq_ps[:], ones8_sb[:], sq[:],
                                     start=(t == 0), stop=(t == NPAIR - 1),
                                     perf_mode=DR)
                # weights/xq land after the squares on the gpsimd queue
                for nm, t in (("q", wq8), ("k", wk8), ("v", wv8), ("p", wp8)):
                    nc.gpsimd.dma_start(w_sb[nm][:], t)
                nc.gpsimd.dma_start(xq8_sb[:], xq8)
                # group means / var / rstd on a [1, 16] row
                inv_n = 1.0 / (32.0 * M)
                mean_row = smal.tile([1, NG], F32, tag="mean_row", bufs=1)
                nc.vector.tensor_reduce(
                    out=mean_row[:],
                    in_=sums_ps[0:1, :].rearrange("o (g j) -> o g j", j=32),
                    axis=mybir.AxisListType.X, op=ALU.add)
                nc.vector.tensor_scalar_mul(out=mean_row[:], in0=mean_row[:],
                                            scalar1=inv_n)
                ex2_row = smal.tile([1, NG], F32, tag="ex2_row", bufs=1)
                nc.vector.tensor_reduce(
                    out=ex2_row[:],
                    in_=sumsq_ps[0:1, :].rearrange("o (g j) -> o g j", j=32),
                    axis=mybir.AxisListType.X, op=ALU.add)
                nc.vector.tensor_scalar_mul(out=ex2_row[:], in0=ex2_row[:],
                                            scalar1=inv_n)
                var_row = smal.tile([1, NG], F32, tag="var_row", bufs=1)
                nc.vector.tensor_tensor(out=var_row[:], in0=mean_row[:],
                                        in1=mean_row[:], op=ALU.mult)
                nc.vector.tensor_sub(var_row[:], ex2_row[:], var_row[:])
                eps_t = smal.tile([1, 1], F32, tag="eps_t", bufs=1)
                nc.vector.memset(eps_t[:], EPS)
                lnv = smal.tile([1, NG], F32, tag="lnv", bufs=1)
                nc.scalar.activation(lnv[:], var_row[:], AF.Ln, bias=eps_t[:])
                rstd_row = smal.tile([1, NG], F32, tag="rstd_row", bufs=1)
                nc.scalar.activation(rstd_row[:], lnv[:], AF.Exp, scale=-0.5)

                # broadcast rows -> [128, 4] channel layout (c = cc*128+p)
                def bcast(src_row, dtag):
                    gvt_ps = ps_x.tile([NG, 1], F32, tag="r")
                    nc.tensor.matmul(gvt_ps[:], src_row[:], one1_sb[:],
                                     start=True, stop=True)
                    gvt = smal.tile([NG, 1], F32, tag=dtag + "_gvt", bufs=1)
                    nc.vector.tensor_copy(gvt[:], gvt_ps[:])
                    mov16 = smal.tile([NG, 4], F32, tag=dtag + "_mov", bufs=1)
                    nc.vector.tensor_scalar_mul(out=mov16[:], in0=gmask_sb[:],
                                                scalar1=gvt[:])
                    bc_ps = ps_r.tile([128, 4], F32, tag="r")
                    nc.tensor.matmul(bc_ps[:], e2_sb[:], mov16[:],
                                     start=True, stop=True)
                    dst = smal.tile([128, 4], F32, tag=dtag, bufs=1)
                    nc.vector.tensor_copy(dst[:], bc_ps[:])
                    return dst

                mean_bc = bcast(mean_row, "mean_bc")
                rstd_bc = bcast(rstd_row, "rstd_bc")
                a_sc = smal.tile([128, 4], F32, tag="a_sc", bufs=1)
                nc.vector.tensor_tensor(out=a_sc[:], in0=cvec["gamma"][:],
                                        in1=rstd_bc[:], op=ALU.mult)
                b_sh = smal.tile([128, 4], F32, tag="b_sh", bufs=1)
                nc.vector.tensor_tensor(out=b_sh[:], in0=a_sc[:],
                                        in1=mean_bc[:], op=ALU.mult)
                nc.vector.tensor_sub(b_sh[:], cvec["beta"][:], b_sh[:])
                b8 = smal.tile([128, 2, 2, 1], FP8, tag="b8", bufs=1)
                nc.vector.tensor_scalar_mul(
                    out=b8[:].rearrange("p a b c -> p (a b c)"), in0=b_sh[:],
                    scalar1=BS)
                a_sck = smal.tile([128, 4], F32, tag="a_sck", bufs=1)
                nc.vector.tensor_scalar_mul(out=a_sck[:], in0=a_sc[:],
                                            scalar1=QKS / WS)
                if _lvl < 1:
                    return

                # ======== Phase W: biases + weight A-folding =============
                def bias_from(wt, bvec, cv, dtag):
                    dst = smal.tile([128, 4], F32, tag=dtag, bufs=1)
                    for oc in range(4):
                        bp_ps = ps_sc.tile([128, 1], F32, tag="sc")
                        for co in range(2):
                            nc.tensor.matmul(
                                bp_ps[:], wt[:, co, :, oc * 128:(oc + 1) * 128],
                                bvec[:, co], start=(co == 0), stop=(co == 1),
                                perf_mode=DR)
                        nc.vector.tensor_scalar(
                            out=dst[:, oc:oc + 1], in0=bp_ps[:],
                            scalar1=1.0 / (WS * BS), scalar2=cv[:, oc:oc + 1],
                            op0=ALU.mult, op1=ALU.add)
                    return dst

                # only bias_q and the Wq fold gate the Q projection;
                # V/P biases and the Wv fold are first needed at qb0's tail
                bias_q = bias_from(w_sb["q"], b8, cvec["bq"], "bias_q")
                for co in range(2):
                    for i in range(2):
                        nc.scalar.activation(
                            out=w_sb["q"][:, co, i, :],
                            in_=w_sb["q"][:, co, i, :], func=AF.Identity,
                            scale=a_sc[:, 2 * co + i:2 * co + i + 1])
                nc.sync.dma_start(xres_sb[:], xrv)

                # ======== Phase Q: queries + QK =========================
                def q_chunk(qh, oc):
                    qsl = slice(qh * QB, (qh + 1) * QB)
                    qp = ps_sc.tile([128, QB], F32, tag="sc")
                    for co in range(2):
                        nc.tensor.matmul(
                            qp[:], w_sb["q"][:, co, :, oc * 128:(oc + 1) * 128],
                            xq8_sb[:, co, :, qsl],
                            start=(co == 0), stop=(co == 1), perf_mode=DR)
                    nc.vector.tensor_scalar(
                        out=q8_sb[:, oc // 2, oc % 2, qsl], in0=qp[:],
                        scalar1=1.0 / WS, scalar2=bias_q[:, oc:oc + 1],
                        op0=ALU.mult, op1=ALU.add)

                def k_chunk(qh, cc):
                    qsl = slice(qh * QB, (qh + 1) * QB)
                    kp = ps_sc.tile([128, QB], F32, tag="sc")
                    for co in range(2):
                        nc.tensor.matmul(
                            kp[:], w_sb["k"][:, co, :, cc * 128:(cc + 1) * 128],
                            q8_sb[:, co, :, qsl],
                            start=(co == 0), stop=(co == 1), perf_mode=DR)
                    nc.vector.tensor_scalar_mul(
                        out=qk8_sb[:, cc // 2, cc % 2, qsl], in0=kp[:],
                        scalar1=a_sck[:, cc:cc + 1])

                # only qh0 gates the qb0 m loop; qh1's production and the
                # V/P bias chain + Wv fold are deferred into qb0's m loop
                for oc in range(4):
                    q_chunk(0, oc)
                for cc in range(4):
                    k_chunk(0, cc)

                vp = {}

                def vp_chain():
                    bv_tot = bias_from(w_sb["v"], b8, cvec["bv"], "bv_tot")
                    b8v = smal.tile([128, 2, 2, 1], FP8, tag="b8v", bufs=1)
                    nc.vector.tensor_scalar_mul(
                        out=b8v[:].rearrange("p a b c -> p (a b c)"),
                        in0=bv_tot[:], scalar1=BS)
                    vp["bias_p"] = bias_from(w_sb["p"], b8v, cvec["bp"],
                                             "bias_p")
                    for co in range(2):
                        for i in range(2):
                            nc.scalar.activation(
                                out=w_sb["v"][:, co, i, :],
                                in_=w_sb["v"][:, co, i, :], func=AF.Identity,
                                scale=a_sc[:, 2 * co + i:2 * co + i + 1])
                if _lvl < 2:
                    return

                # ======== Phase B: m loop (scores, exp, PV, r) ==========
                def m_loop(qb, pre_work):
                    qsl = slice(qb * QB, (qb + 1) * QB)
                    ho_ps = ps_ho.tile([128, 4, QB], F32, tag="ho",
                                       name=f"ho{qb}")
                    r_ps = ps_r.tile([128, QB], F32, tag="r", name=f"r{qb}")
                    p_tiles = {}

                    def scores_step(t):
                        p_pair = pgr.tile([128, 2, QB], FP8, tag="p",
                                          name=f"p{qb}_{t}")
                        for i in range(2):
                            mt = 2 * t + i
                            sc = ps_sc.tile([128, QB], F32, tag="sc",
                                            name=f"sc{qb}_{mt}")
                            for co in range(2):
                                nc.tensor.matmul(
                                    sc[:],
                                    x8_sb[:, co, :, mt * 128:(mt + 1) * 128],
                                    qk8_sb[:, co, :, qsl],
                                    start=(co == 0), stop=(co == 1),
                                    perf_mode=DR)
                            nc.scalar.activation(out=p_pair[:, i, :], in_=sc[:],
                                                 func=AF.Exp, scale=SCALE / QKS)
                        p_tiles[t] = p_pair

                    def pv_step(t):
                        p_pair = p_tiles.pop(t)
                        first, last = t == 0, t == NPAIR - 1
                        for oc in range(4):
                            nc.tensor.matmul(
                                ho_ps[:, oc, :],
                                xt8_sb[:, t, :, oc * 128:(oc + 1) * 128],
                                p_pair[:], start=first, stop=last,
                                perf_mode=DR)
                        nc.tensor.matmul(r_ps[:], ones8_sb[:], p_pair[:],
                                         start=first, stop=last, perf_mode=DR)

                    scores_step(0)
                    for t in range(1, NPAIR):
                        scores_step(t)
                        pv_step(t - 1)
                        if t - 1 < len(pre_work):
                            pre_work[t - 1]()
                    pv_step(NPAIR - 1)
                    for f in pre_work[NPAIR - 1:]:
                        f()
                    return ho_ps, r_ps

                def early_tail(qb, ho_ps, r_ps):
                    """normalize ho -> fp8; returns deferred Vfold/proj work"""
                    qsl = slice(qb * QB, (qb + 1) * QB)
                    r_sb = smal.tile([1, QB], F32, tag="r_sb", bufs=2,
                                     name=f"rsb{qb}")
                    nc.vector.tensor_scalar_mul(out=r_sb[:], in0=r_ps[0:1, :],
                                                scalar1=1.0 / HOS)
                    ibr_row = smal.tile([1, QB], F32, tag="ibr_row", bufs=2,
                                        name=f"ibr{qb}")
                    nc.vector.reciprocal(ibr_row[:], r_sb[:])
                    ibr_ps = ps_x.tile([128, QB], F32, tag="r")
                    nc.tensor.matmul(ibr_ps[:], ones1_sb[:], ibr_row[:],
                                     start=True, stop=True)
                    ibr_sb = smal.tile([128, QB], F32, tag="ibr_sb", bufs=2,
                                       name=f"ibrsb{qb}")
                    nc.vector.tensor_copy(ibr_sb[:], ibr_ps[:])
                    ho8 = hop.tile([128, 2, 2, QB], FP8, tag="ho8",
                                   name=f"ho8_{qb}")
                    for oc in range(4):
                        nc.vector.tensor_tensor(
                            out=ho8[:, oc // 2, oc % 2, :],
                            in0=ho_ps[:, oc, :], in1=ibr_sb[:], op=ALU.mult)
                    hv8 = hop.tile([128, 2, 2, QB], FP8, tag="ho8",
                                   name=f"hv8_{qb}")
                    late = []

                    def hv_chunk(vc, qb=qb):
                        hp = ps_sc.tile([128, QB], F32, tag="sc")
                        for co in range(2):
                            nc.tensor.matmul(
                                hp[:], w_sb["v"][:, co, :, vc * 128:(vc + 1) * 128],
                                ho8[:, co], start=(co == 0), stop=(co == 1),
                                perf_mode=DR)
                        nc.scalar.activation(
                            out=hv8[:, vc // 2, vc % 2, :], in_=hp[:],
                            func=AF.Identity, scale=HVS / (WS * HOS))

                    def pj_chunk(oc, qb=qb, qsl=qsl):
                        pj = ps_sc.tile([128, QB], F32, tag="sc")
                        for co in range(2):
                            nc.tensor.matmul(
                                pj[:], w_sb["p"][:, co, :, oc * 128:(oc + 1) * 128],
                                hv8[:, co], start=(co == 0), stop=(co == 1),
                                perf_mode=DR)
                        o_sb = dmy.tile([128, QB], F32, tag="osb",
                                        name=f"osb{qb}_{oc}")
                        nc.vector.scalar_tensor_tensor(
                            out=o_sb[:], in0=pj[:], scalar=1.0 / (WS * HVS),
                            in1=xres_sb[:, oc, qsl], op0=ALU.mult, op1=ALU.add)
                        nc.sync.dma_start(outv[:, oc, qsl], o_sb[:])

                    for vc in range(4):
                        late.append(lambda vc=vc: hv_chunk(vc))
                    for oc in range(4):
                        late.append(lambda oc=oc: pj_chunk(oc))
                    return late

                late = ([lambda oc=oc: q_chunk(1, oc) for oc in range(4)]
                        + [lambda cc=cc: k_chunk(1, cc) for cc in range(4)]
                        + [vp_chain])
                for qb in range(NQB):
                    ho_ps, r_ps = m_loop(qb, late)
                    if _lvl < 3:
                        return
                    late = early_tail(qb, ho_ps, r_ps)
                    if qb == 0:
                        # residual + output-bias staging (in place); needed
                        # first by qb0's deferred proj chunks, which run
                        # inside qb1's m loop -- emit here, off the
                        # pre-loop critical path
                        for oc in range(4):
                            nc.vector.tensor_scalar_add(
                                out=xres_sb[:, oc], in0=xres_sb[:, oc],
                                scalar1=vp["bias_p"][:, oc:oc + 1])
                for f in late:
                    f()

            if reps == 1:
                body()
            else:
                with tc.For_i(0, reps, 1):
                    body()

    nc.compile()
    return nc


def make_in_maps(x, gamma, beta, Wq, bq, Wk, bk, Wv, bv, Wp, bp):
    x2d = np.ascontiguousarray(np.asarray(x, dtype=np.float32).reshape(C, M))

    def clayout(a):  # [C, F] -> [128, 2, 2, F], c = co*256+i*128+p
        return np.ascontiguousarray(
            a.reshape(2, 2, 128, -1).transpose(2, 0, 1, 3))

    Wq = np.asarray(Wq, np.float32)
    Wk = np.asarray(Wk, np.float32)
    Wv = np.asarray(Wv, np.float32)
    Wp = np.asarray(Wp, np.float32)
    xt = np.ascontiguousarray(x2d.T).reshape(NPAIR, 2, 128, C)
    consts = {
        "x8": clayout(x2d).astype(E4),
        "xt8": np.ascontiguousarray(xt.transpose(2, 0, 1, 3)).astype(E4),
        "wq8": clayout(Wq.T * WS).astype(E4),
        "wk8": clayout(Wk * WS).astype(E4),
        "wv8": clayout(Wv.T * WS).astype(E4),
        "wp8": clayout(Wp.T * WS).astype(E4),
        "bq": np.asarray(bq, np.float32),
        "bv": np.asarray(bv, np.float32),
        "bp": np.asarray(bp, np.float32),
        "gamma": np.asarray(gamma, np.float32),
        "beta": np.asarray(beta, np.float32),
        "ones8": np.ones((128, 2, 128), E4),
        "one1": np.ones((1, 1), np.float32),
        "ones_1": np.ones((1, 128), np.float32),
        "e2": np.equal(np.arange(16)[:, None] % 4,
                       np.arange(128)[None, :] // 32).astype(np.float32),
        "gmask": np.equal(np.arange(16)[:, None] // 4,
                          np.arange(4)[None, :]).astype(np.float32),
    }
    in_maps = []
    for i in range(N_CORES):
        sl = x2d[:, i * QS:(i + 1) * QS]
        m = dict(consts)
        m["xq8"] = clayout(sl).astype(E4)
        m["x_res"] = np.ascontiguousarray(sl)
        in_maps.append(m)
    return in_maps


_NC_CACHE = {}


def get_nc(reps=1):
    if reps not in _NC_CACHE:
        _NC_CACHE[reps] = build_nc(reps)
    return _NC_CACHE[reps]


def kernel(**inputs):
    in_maps = make_in_maps(**inputs)
    nc = get_nc(1)
    res = run_bass_kernel_spmd(nc, in_maps, core_ids=list(range(N_CORES)))
    full = np.concatenate([res.results[i]["out"] for i in range(N_CORES)],
                          axis=1)
    return full.reshape(1, C, 8, 32, 32).astype(np.float32)


if __name__ == "__main__":
    import time
    t0 = time.time()
    nc = build_nc(1)
    print(f"build: {time.time()-t0:.1f}s")
